# revision 3
# baseline (speedup 1.0000x reference)
"""Trainium2 (8 NeuronCores) kernel for a dense causal multi-head attention block.

Problem shapes: B=2, S=2048, D=2048, H=16, DH=128 (fp32 in/out).

Distribution strategy (sharding_hint: tensor-parallel over heads):
  Phase 1 (head parallel): core c owns heads {2c, 2c+1}. It computes, for both
    batches, Q^T/K^T/V^T = W^T @ X^T directly in [DH, S] layout (lhsT = W tile,
    rhs = X^T tile loaded via XBAR DMA-transpose of the bf16 input), then causal
    attention fully on-chip:
       scores^T[k, q] = K^T.T @ Q^T          (PE, one matmul per 128x512 tile)
       p = exp(scores / sqrt(DH))            (ACT, straight from PSUM; no
                                              max-subtraction -- scores are O(1))
       diagonal tiles masked by a 0/1 bf16 mask (DVE)
       z^T[dh, q]  += V_tile.T @ p           (PE, PSUM accumulation over k)
       den[1, q]   += ones.T @ p             (PE rank-reduce for softmax denom)
       z^T *= 1/den (DVE recip + gpsimd partition_broadcast + DVE mult)
  AllToAll (2 MB bf16): reshards z^T from (head-sharded, all rows) to
    (all heads, 512-row shard) so each core owns rows of the final output.
  Phase 2 (row parallel): out[q, d] = Z^T.T @ W_O + b_O for the core's 512 rows.

The host wrapper shards/casts inputs (bf16), runs the SPMD NEFF on cores 0-7,
and concatenates the per-core row slices into the full [2, 2048, 2048] output.
"""

import numpy as np
import ml_dtypes

import concourse.bass as bass
import concourse.mybir as mybir
import concourse.tile as tile
from concourse import bacc
from concourse.bass import ts
from concourse.bass_utils import run_bass_kernel_spmd
from concourse.masks import make_identity

B, S, D, H, DH = 2, 2048, 2048, 16, 128
NCORES = 8
HL = H // NCORES            # heads per core = 2
QB = (B * S) // NCORES      # output rows per core = 512
P = 128
SC = 512                    # free-dim chunk (PSUM bank = 512 fp32)
NSC = S // SC               # 4
NDT = D // P                # 16 contraction tiles for D
NST = S // P                # 16 sequence tiles of 128
NQT = QB // P               # 4 local q tiles in phase 2
NDC = D // SC               # 4 output-dim chunks
SCALE = 1.0 / float(np.sqrt(DH))
MASKW = 384 + SC            # shifted-triangle mask width

F32 = mybir.dt.float32
BF16 = mybir.dt.bfloat16


def build_nc():
    nc = bacc.Bacc("TRN2", target_bir_lowering=False, debug=False,
                   num_devices=NCORES)

    x = nc.dram_tensor("x", [B, S, D], BF16, kind="ExternalInput")
    wq = nc.dram_tensor("wq", [HL, D, DH], BF16, kind="ExternalInput")
    wk = nc.dram_tensor("wk", [HL, D, DH], BF16, kind="ExternalInput")
    wv = nc.dram_tensor("wv", [HL, D, DH], BF16, kind="ExternalInput")
    bq = nc.dram_tensor("bq", [HL, DH], F32, kind="ExternalInput")
    bk = nc.dram_tensor("bk", [HL, DH], F32, kind="ExternalInput")
    bv = nc.dram_tensor("bv", [HL, DH], F32, kind="ExternalInput")
    wo = nc.dram_tensor("wo", [H * DH, D], BF16, kind="ExternalInput")
    bo = nc.dram_tensor("bo", [1, D], BF16, kind="ExternalInput")
    out = nc.dram_tensor("out", [QB, D], F32, kind="ExternalOutput")

    Exp = mybir.ActivationFunctionType.Exp
    Ident = mybir.ActivationFunctionType.Identity

    with tile.TileContext(nc) as tc:
        with (
            tc.tile_pool(name="const", bufs=1) as cpool,
            tc.tile_pool(name="dram", bufs=1, space="DRAM") as dpool,
            tc.tile_pool(name="ps_acc", bufs=2, space="PSUM") as ps_acc,
            tc.tile_pool(name="ps_z", bufs=2, space="PSUM") as ps_z,
            tc.tile_pool(name="ps_vt", bufs=2, space="PSUM") as ps_vt,
            tc.tile_pool(name="ps_den", bufs=2, space="PSUM") as ps_den,
        ):
            # ---- constants ----
            ident = cpool.tile([P, P], BF16)
            make_identity(nc, ident)
            ones_col = cpool.tile([P, 1], BF16)
            nc.vector.memset(ones_col, 1.0)
            ones_row = cpool.tile([1, P], BF16)
            nc.vector.memset(ones_row, 1.0)
            # mask[ki, t] = 1.0 iff ki <= t - 384; slices give the 4 shifted
            # causal triangles needed for the diagonal 128x512 tiles.
            mask = cpool.tile([P, MASKW], BF16)
            nc.gpsimd.memset(mask, 1.0)
            nc.gpsimd.affine_select(
                out=mask, in_=mask, compare_op=mybir.AluOpType.is_ge,
                fill=0.0, base=-384, pattern=[[1, MASKW]], channel_multiplier=-1,
            )
            bias_sb = {}
            for nm, t in (("q", bq), ("k", bk), ("v", bv)):
                bb = cpool.tile([P, HL], F32, tag=f"b{nm}")
                nc.sync.dma_start(bb, t.ap().rearrange("h d -> d h"))
                bias_sb[nm] = bb
            bo_sb = cpool.tile([1, D], BF16)
            nc.sync.dma_start(bo_sb, bo.ap())

            a2a_in = dpool.tile([NCORES, HL, P, SC], BF16, tag="a2a_in")
            a2a_out = dpool.tile([NCORES, HL, P, SC], BF16, tag="a2a_out")

            with (
                tc.tile_pool(name="wpool", bufs=1) as wpool,
                tc.tile_pool(name="xt", bufs=1) as xtpool,
                tc.tile_pool(name="qkv", bufs=2) as qkvpool,
                tc.tile_pool(name="small", bufs=4) as spool,
            ):
                # per-head weight tiles [d_part, d_tile, dh]
                w_sb = []
                for hl in range(HL):
                    per = []
                    for nm, w in (("wq", wq), ("wk", wk), ("wv", wv)):
                        t_sb = wpool.tile([P, NDT, DH], BF16, tag=f"{nm}{hl}")
                        nc.sync.dma_start(
                            t_sb, w.ap()[hl].rearrange("(o p) k -> p o k", p=P))
                        per.append(t_sb)
                    w_sb.append(per)

                for b in range(B):
                    # X^T for this batch: [d_part, d_tile, s] via DMA transpose
                    XT = xtpool.tile([P, NDT, S], BF16, tag="xt")
                    for dt_ in range(NDT):
                        nc.sync.dma_start_transpose(
                            XT[:, dt_, :], x.ap()[b][:, ts(dt_, P)])

                    for hl in range(HL):
                        # ---- projections: Q^T, K^T, V^T in [dh, s] ----
                        QT = qkvpool.tile([P, S], BF16, tag="qt")
                        KT = qkvpool.tile([P, S], BF16, tag="kt")
                        VT = qkvpool.tile([P, S], BF16, tag="vt")
                        for pi, (dst, bcol) in enumerate((
                            (QT, bias_sb["q"]), (KT, bias_sb["k"]),
                            (VT, bias_sb["v"]),
                        )):
                            wt = w_sb[hl][pi]
                            for sc in range(NSC):
                                ps = ps_acc.tile([P, SC], F32, tag="acc")
                                for dt_ in range(NDT):
                                    nc.tensor.matmul(
                                        ps, lhsT=wt[:, dt_, :],
                                        rhs=XT[:, dt_, ts(sc, SC)],
                                        start=(dt_ == 0), stop=(dt_ == NDT - 1))
                                nc.scalar.activation(
                                    dst[:, ts(sc, SC)], ps, Ident,
                                    bias=bcol[:, hl:hl + 1], scale=1.0)

                        # ---- V in [k, dh] layout via PE transpose ----
                        V_kd = qkvpool.tile([P, NST, DH], BF16, tag="vkd")
                        for st in range(NST):
                            pst = ps_vt.tile([P, P], BF16, tag="vt")
                            nc.tensor.transpose(pst, VT[:, ts(st, P)], ident)
                            nc.scalar.copy(V_kd[:, st, :], pst)

                        # ---- causal attention ----
                        for qc in range(NSC):
                            z_ps = ps_z.tile([P, SC], F32, tag="z")
                            den_ps = ps_den.tile([1, SC], F32, tag="den")
                            nkt = 4 * qc + 4
                            for kt in range(nkt):
                                s_ps = ps_acc.tile([P, SC], F32, tag="acc")
                                nc.tensor.matmul(
                                    s_ps, lhsT=KT[:, ts(kt, P)],
                                    rhs=QT[:, ts(qc, SC)],
                                    start=True, stop=True)
                                pexp = spool.tile([P, SC], BF16, tag="p")
                                nc.scalar.activation(
                                    pexp, s_ps, Exp, bias=0.0, scale=SCALE)
                                j = kt - 4 * qc
                                if j >= 0:
                                    off = 384 - 128 * j
                                    nc.vector.tensor_mul(
                                        pexp, pexp, mask[:, off:off + SC])
                                nc.tensor.matmul(
                                    z_ps, lhsT=V_kd[:, kt, :], rhs=pexp,
                                    start=(kt == 0), stop=(kt == nkt - 1))
                                nc.tensor.matmul(
                                    den_ps, lhsT=ones_col, rhs=pexp,
                                    start=(kt == 0), stop=(kt == nkt - 1))
                            # normalize: z^T * (1/den) broadcast over partitions
                            rden = spool.tile([1, SC], F32, tag="rden")
                            nc.vector.reciprocal(rden, den_ps)
                            rb = spool.tile([P, SC], F32, tag="rb", bufs=2)
                            nc.gpsimd.partition_broadcast(rb, rden)
                            zs = spool.tile([P, SC], BF16, tag="zs", bufs=2)
                            nc.vector.tensor_mul(zs, z_ps, rb)
                            nc.sync.dma_start(a2a_in[4 * b + qc, hl], zs)

            # ---- reshard z: (heads local, all rows) -> (all heads, rows local)
            nc.gpsimd.collective_compute(
                "AllToAll", mybir.AluOpType.bypass,
                replica_groups=[list(range(NCORES))],
                ins=[a2a_in[:]], outs=[a2a_out[:]],
            )

            # ---- phase 2: output projection for this core's 512 rows ----
            with (
                tc.tile_pool(name="p2", bufs=1) as p2pool,
                tc.tile_pool(name="p2o", bufs=2) as p2opool,
            ):
                WO_sb = p2pool.tile([P, H, D], BF16, tag="wo")
                for t in range(H):
                    nc.sync.dma_start(WO_sb[:, t, :], wo.ap()[ts(t, P), :])
                ZT_sb = p2pool.tile([P, H, SC], BF16, tag="zt")
                for j in range(NCORES):
                    for hl in range(HL):
                        nc.sync.dma_start(ZT_sb[:, 2 * j + hl, :], a2a_out[j, hl])
                for qt in range(NQT):
                    for dc in range(NDC):
                        ops = ps_acc.tile([P, SC], F32, tag="acc")
                        for t in range(H):
                            nc.tensor.matmul(
                                ops, lhsT=ZT_sb[:, t, ts(qt, P)],
                                rhs=WO_sb[:, t, ts(dc, SC)],
                                start=(t == 0), stop=False)
                        nc.tensor.matmul(
                            ops, lhsT=ones_row, rhs=bo_sb[:, ts(dc, SC)],
                            start=False, stop=True)
                        osb = p2opool.tile([P, SC], F32, tag="osb")
                        nc.scalar.copy(osb, ops)
                        nc.sync.dma_start(out.ap()[ts(qt, P), ts(dc, SC)], osb)

    nc.compile()
    return nc


_CACHE = {}


def _get_nc():
    if "nc" not in _CACHE:
        _CACHE["nc"] = build_nc()
    return _CACHE["nc"]


def make_in_maps(resid_pre, W_Q, W_K, W_V, W_O, b_Q, b_K, b_V, b_O):
    bf = ml_dtypes.bfloat16
    x = np.ascontiguousarray(np.asarray(resid_pre, np.float32)).astype(bf)
    WQ = np.asarray(W_Q, np.float32)
    WK = np.asarray(W_K, np.float32)
    WV = np.asarray(W_V, np.float32)
    WOf = np.ascontiguousarray(
        np.asarray(W_O, np.float32).reshape(H * DH, D)).astype(bf)
    bQ = np.ascontiguousarray(np.asarray(b_Q, np.float32))
    bK = np.ascontiguousarray(np.asarray(b_K, np.float32))
    bV = np.ascontiguousarray(np.asarray(b_V, np.float32))
    bO = np.ascontiguousarray(np.asarray(b_O, np.float32)).reshape(1, D).astype(bf)
    in_maps = []
    for c in range(NCORES):
        hs = slice(c * HL, (c + 1) * HL)
        in_maps.append({
            "x": x,
            "wq": np.ascontiguousarray(WQ[hs]).astype(bf),
            "wk": np.ascontiguousarray(WK[hs]).astype(bf),
            "wv": np.ascontiguousarray(WV[hs]).astype(bf),
            "bq": bQ[hs].copy(),
            "bk": bK[hs].copy(),
            "bv": bV[hs].copy(),
            "wo": WOf,
            "bo": bO,
        })
    return in_maps


def assemble(results):
    out = np.empty((B, S, D), np.float32)
    for c in range(NCORES):
        b, r = divmod(c, NCORES // B)  # divmod(c, 4)
        out[b, r * QB:(r + 1) * QB] = results[c]["out"]
    return out


def kernel(resid_pre, W_Q, W_K, W_V, W_O, b_Q, b_K, b_V, b_O,
           _trace=False, _return_raw=False):
    nc = _get_nc()
    in_maps = make_in_maps(resid_pre, W_Q, W_K, W_V, W_O, b_Q, b_K, b_V, b_O)
    res = run_bass_kernel_spmd(nc, in_maps, core_ids=list(range(NCORES)),
                               trace=_trace)
    out = assemble(res.results)
    if _return_raw:
        return out, res
    return out


# revision 10
# speedup vs baseline: 1.1115x; 1.1115x over previous
"""Trainium2 (8 NeuronCores) kernel for a dense causal multi-head attention block.

Problem shapes: B=2, S=2048, D=2048, H=16, DH=128 (fp32 in/out).

Distribution strategy (sharding_hint: tensor-parallel over heads):
  Phase 1 (head parallel): core c owns heads {2c, 2c+1}. It computes, for both
    batches, Q^T/K^T/V^T = W^T @ X^T directly in [DH, S] layout (lhsT = W tile,
    rhs = X^T tile loaded via XBAR DMA-transpose of the bf16 input), then causal
    attention fully on-chip:
       scores^T[k, q] = K^T.T @ Q^T          (PE, one matmul per 128x512 tile)
       p = exp(scores / sqrt(DH))            (ACT, straight from PSUM; no
                                              max-subtraction -- scores are O(1))
       diagonal tiles masked by a 0/1 bf16 mask (DVE)
       z^T[dh, q]  += V_tile.T @ p           (PE, PSUM accumulation over k)
       den[1, q]   += ones.T @ p             (PE rank-reduce for softmax denom)
       z^T *= 1/den (DVE recip + gpsimd partition_broadcast + DVE mult)
  AllToAll (2 MB bf16): reshards z^T from (head-sharded, all rows) to
    (all heads, 512-row shard) so each core owns rows of the final output.
  Phase 2 (row parallel): out[q, d] = Z^T.T @ W_O + b_O for the core's 512 rows.

The host wrapper shards/casts inputs (bf16), runs the SPMD NEFF on cores 0-7,
and concatenates the per-core row slices into the full [2, 2048, 2048] output.
"""

import numpy as np
import ml_dtypes

import concourse.bass as bass
import concourse.mybir as mybir
import concourse.tile as tile
from concourse import bacc
from concourse.bass import ts
from concourse.bass_utils import run_bass_kernel_spmd
from concourse.masks import make_identity

B, S, D, H, DH = 2, 2048, 2048, 16, 128
NCORES = 8
HL = H // NCORES            # heads per core = 2
QB = (B * S) // NCORES      # output rows per core = 512
P = 128
SC = 512                    # free-dim chunk (PSUM bank = 512 fp32)
NSC = S // SC               # 4
NDT = D // P                # 16 contraction tiles for D
NST = S // P                # 16 sequence tiles of 128
NQT = QB // P               # 4 local q tiles in phase 2
NDC = D // SC               # 4 output-dim chunks
SCALE = 1.0 / float(np.sqrt(DH))
MASKW = 384 + SC            # shifted-triangle mask width

F32 = mybir.dt.float32
BF16 = mybir.dt.bfloat16


def build_nc():
    nc = bacc.Bacc("TRN2", target_bir_lowering=False, debug=False,
                   num_devices=NCORES)

    x = nc.dram_tensor("x", [B, S, D], BF16, kind="ExternalInput")
    wq = nc.dram_tensor("wq", [HL, D, DH], BF16, kind="ExternalInput")
    wk = nc.dram_tensor("wk", [HL, D, DH], BF16, kind="ExternalInput")
    wv = nc.dram_tensor("wv", [HL, D, DH], BF16, kind="ExternalInput")
    bq = nc.dram_tensor("bq", [HL, DH], F32, kind="ExternalInput")
    bk = nc.dram_tensor("bk", [HL, DH], F32, kind="ExternalInput")
    bv = nc.dram_tensor("bv", [HL, DH], F32, kind="ExternalInput")
    wo = nc.dram_tensor("wo", [H * DH, D], BF16, kind="ExternalInput")
    bo = nc.dram_tensor("bo", [1, D], BF16, kind="ExternalInput")
    out = nc.dram_tensor("out", [QB, D], F32, kind="ExternalOutput")

    Exp = mybir.ActivationFunctionType.Exp
    Ident = mybir.ActivationFunctionType.Identity

    with tile.TileContext(nc) as tc:
        with (
            tc.tile_pool(name="const", bufs=1) as cpool,
            tc.tile_pool(name="dram", bufs=1, space="DRAM") as dpool,
            tc.tile_pool(name="ps_acc", bufs=3, space="PSUM") as ps_acc,
            tc.tile_pool(name="ps_z", bufs=2, space="PSUM") as ps_z,
            tc.tile_pool(name="ps_vt", bufs=2, space="PSUM") as ps_vt,
            tc.tile_pool(name="ps_den", bufs=1, space="PSUM") as ps_den,
        ):
            # ---- constants ----
            ident = cpool.tile([P, P], BF16)
            make_identity(nc, ident)
            ones_col = cpool.tile([P, 1], BF16)
            nc.vector.memset(ones_col, 1.0)
            ones_row = cpool.tile([1, P], BF16)
            nc.vector.memset(ones_row, 1.0)
            # mask[ki, t] = 1.0 iff ki <= t - 384; slices give the 4 shifted
            # causal triangles needed for the diagonal 128x512 tiles.
            mask = cpool.tile([P, MASKW], BF16)
            nc.gpsimd.memset(mask, 1.0)
            nc.gpsimd.affine_select(
                out=mask, in_=mask, compare_op=mybir.AluOpType.is_ge,
                fill=0.0, base=-384, pattern=[[1, MASKW]], channel_multiplier=-1,
            )
            bias_sb = {}
            for nm, t in (("q", bq), ("k", bk), ("v", bv)):
                bb = cpool.tile([P, HL], F32, tag=f"b{nm}")
                nc.sync.dma_start(bb, t.ap().rearrange("h d -> d h"))
                bias_sb[nm] = bb
            bo_sb = cpool.tile([1, D], BF16)
            nc.sync.dma_start(bo_sb, bo.ap())

            # one AllToAll per local head index: the first launches halfway
            # through phase 1 and hides under compute of the second head.
            a2a_in = [dpool.tile([NCORES, P, SC], BF16, tag=f"a2a_in{hl}",
                                 name=f"a2a_in{hl}") for hl in range(HL)]
            a2a_out = [dpool.tile([NCORES, P, SC], BF16, tag=f"a2a_out{hl}",
                                  name=f"a2a_out{hl}") for hl in range(HL)]

            with (
                tc.tile_pool(name="wpool", bufs=1) as wpool,
                tc.tile_pool(name="xt", bufs=1) as xtpool,
                tc.tile_pool(name="qkv", bufs=2) as qkvpool,
                tc.tile_pool(name="small", bufs=4) as spool,
            ):
                # per-head weight tiles [d_part, d_tile, dh]
                w_sb = []
                for hl in range(HL):
                    per = []
                    for nm, w in (("wq", wq), ("wk", wk), ("wv", wv)):
                        t_sb = wpool.tile([P, NDT, DH], BF16, tag=f"{nm}{hl}")
                        nc.sync.dma_start(
                            t_sb, w.ap()[hl].rearrange("(o p) k -> p o k", p=P))
                        per.append(t_sb)
                    w_sb.append(per)

                XT = {}
                for hl in range(HL):
                    for b in range(B):
                        if hl == 0:
                            # X^T for batch b: [d_part, d_tile, s] (DMA xpose)
                            XT[b] = xtpool.tile([P, NDT, S], BF16,
                                                tag=f"xt{b}", name=f"xt{b}")
                            for dt_ in range(NDT):
                                nc.sync.dma_start_transpose(
                                    XT[b][:, dt_, :], x.ap()[b][:, ts(dt_, P)])

                        # ---- projections: Q^T, K^T, V^T in [dh, s] ----
                        QT = qkvpool.tile([P, S], BF16, tag="qt")
                        KT = qkvpool.tile([P, S], BF16, tag="kt")
                        VT = qkvpool.tile([P, S], BF16, tag="vt")
                        for pi, (dst, bcol) in enumerate((
                            (QT, bias_sb["q"]), (KT, bias_sb["k"]),
                            (VT, bias_sb["v"]),
                        )):
                            wt = w_sb[hl][pi]
                            for sc in range(NSC):
                                ps = ps_acc.tile([P, SC], F32, tag="acc")
                                for dt_ in range(NDT):
                                    nc.tensor.matmul(
                                        ps, lhsT=wt[:, dt_, :],
                                        rhs=XT[b][:, dt_, ts(sc, SC)],
                                        start=(dt_ == 0), stop=(dt_ == NDT - 1))
                                nc.scalar.activation(
                                    dst[:, ts(sc, SC)], ps, Ident,
                                    bias=bcol[:, hl:hl + 1], scale=1.0)

                        # ---- V in [k, dh] layout via PE transpose ----
                        V_kd = qkvpool.tile([P, NST, DH], BF16, tag="vkd")
                        for st in range(NST):
                            pst = ps_vt.tile([P, P], BF16, tag="vt")
                            nc.tensor.transpose(pst, VT[:, ts(st, P)], ident)
                            nc.scalar.copy(V_kd[:, st, :], pst)

                        # ---- causal attention (scores pipelined 2 ahead so
                        # PE never stalls on ACT exp / DVE mask) ----
                        for qc in range(NSC):
                            z_ps = ps_z.tile([P, SC], F32, tag="z")
                            den_ps = ps_den.tile([1, SC], F32, tag="den")
                            nkt = 4 * qc + 4
                            pexps = {}

                            def emit_scores(kt, qc=qc, pexps=None):
                                s_ps = ps_acc.tile([P, SC], F32, tag="acc")
                                nc.tensor.matmul(
                                    s_ps, lhsT=KT[:, ts(kt, P)],
                                    rhs=QT[:, ts(qc, SC)],
                                    start=True, stop=True)
                                pexp = spool.tile([P, SC], BF16, tag="p")
                                nc.scalar.activation(
                                    pexp, s_ps, Exp, bias=0.0, scale=SCALE)
                                j = kt - 4 * qc
                                if j >= 0:
                                    off = 384 - 128 * j
                                    nc.vector.tensor_mul(
                                        pexp, pexp, mask[:, off:off + SC])
                                pexps[kt] = pexp

                            emit_scores(0, pexps=pexps)
                            if nkt > 1:
                                emit_scores(1, pexps=pexps)
                            for kt in range(nkt):
                                if kt + 2 < nkt:
                                    emit_scores(kt + 2, pexps=pexps)
                                pexp = pexps.pop(kt)
                                nc.tensor.matmul(
                                    z_ps, lhsT=V_kd[:, kt, :], rhs=pexp,
                                    start=(kt == 0), stop=(kt == nkt - 1))
                                nc.tensor.matmul(
                                    den_ps, lhsT=ones_col, rhs=pexp,
                                    start=(kt == 0), stop=(kt == nkt - 1))
                            # normalize: z^T * (1/den) broadcast over partitions
                            rden = spool.tile([1, SC], F32, tag="rden", bufs=2)
                            nc.vector.reciprocal(rden, den_ps)
                            rb = spool.tile([P, SC], F32, tag="rb", bufs=2)
                            nc.gpsimd.partition_broadcast(rb, rden)
                            zs = spool.tile([P, SC], BF16, tag="zs", bufs=2)
                            nc.vector.tensor_mul(zs, z_ps, rb)
                            nc.sync.dma_start(a2a_in[hl][4 * b + qc], zs)

                    # reshard this head's z: all (b, qc) chunks are now queued
                    nc.gpsimd.collective_compute(
                        "AllToAll", mybir.AluOpType.bypass,
                        replica_groups=[list(range(NCORES))],
                        ins=[a2a_in[hl][:]], outs=[a2a_out[hl][:]],
                    )

            # ---- phase 2: output projection for this core's 512 rows ----
            with (
                tc.tile_pool(name="p2", bufs=1) as p2pool,
                tc.tile_pool(name="p2o", bufs=2) as p2opool,
            ):
                WO_sb = p2pool.tile([P, H, D], BF16, tag="wo")
                for t in range(H):
                    nc.sync.dma_start(WO_sb[:, t, :], wo.ap()[ts(t, P), :])
                ZT_sb = p2pool.tile([P, H, SC], BF16, tag="zt")
                for j in range(NCORES):
                    for hl in range(HL):
                        nc.sync.dma_start(ZT_sb[:, 2 * j + hl, :],
                                          a2a_out[hl][j])
                for qt in range(NQT):
                    for dc in range(NDC):
                        ops = ps_acc.tile([P, SC], F32, tag="acc")
                        for t in range(H):
                            nc.tensor.matmul(
                                ops, lhsT=ZT_sb[:, t, ts(qt, P)],
                                rhs=WO_sb[:, t, ts(dc, SC)],
                                start=(t == 0), stop=False)
                        nc.tensor.matmul(
                            ops, lhsT=ones_row, rhs=bo_sb[:, ts(dc, SC)],
                            start=False, stop=True)
                        osb = p2opool.tile([P, SC], F32, tag="osb")
                        nc.scalar.copy(osb, ops)
                        nc.sync.dma_start(out.ap()[ts(qt, P), ts(dc, SC)], osb)

    nc.compile()
    return nc


_CACHE = {}


def _get_nc():
    if "nc" not in _CACHE:
        _CACHE["nc"] = build_nc()
    return _CACHE["nc"]


def make_in_maps(resid_pre, W_Q, W_K, W_V, W_O, b_Q, b_K, b_V, b_O):
    bf = ml_dtypes.bfloat16
    x = np.ascontiguousarray(np.asarray(resid_pre, np.float32)).astype(bf)
    WQ = np.asarray(W_Q, np.float32)
    WK = np.asarray(W_K, np.float32)
    WV = np.asarray(W_V, np.float32)
    WOf = np.ascontiguousarray(
        np.asarray(W_O, np.float32).reshape(H * DH, D)).astype(bf)
    bQ = np.ascontiguousarray(np.asarray(b_Q, np.float32))
    bK = np.ascontiguousarray(np.asarray(b_K, np.float32))
    bV = np.ascontiguousarray(np.asarray(b_V, np.float32))
    bO = np.ascontiguousarray(np.asarray(b_O, np.float32)).reshape(1, D).astype(bf)
    in_maps = []
    for c in range(NCORES):
        hs = slice(c * HL, (c + 1) * HL)
        in_maps.append({
            "x": x,
            "wq": np.ascontiguousarray(WQ[hs]).astype(bf),
            "wk": np.ascontiguousarray(WK[hs]).astype(bf),
            "wv": np.ascontiguousarray(WV[hs]).astype(bf),
            "bq": bQ[hs].copy(),
            "bk": bK[hs].copy(),
            "bv": bV[hs].copy(),
            "wo": WOf,
            "bo": bO,
        })
    return in_maps


def assemble(results):
    out = np.empty((B, S, D), np.float32)
    for c in range(NCORES):
        b, r = divmod(c, NCORES // B)  # divmod(c, 4)
        out[b, r * QB:(r + 1) * QB] = results[c]["out"]
    return out


def kernel(resid_pre, W_Q, W_K, W_V, W_O, b_Q, b_K, b_V, b_O,
           _trace=False, _return_raw=False):
    nc = _get_nc()
    in_maps = make_in_maps(resid_pre, W_Q, W_K, W_V, W_O, b_Q, b_K, b_V, b_O)
    res = run_bass_kernel_spmd(nc, in_maps, core_ids=list(range(NCORES)),
                               trace=_trace)
    out = assemble(res.results)
    if _return_raw:
        return out, res
    return out


# revision 13
# speedup vs baseline: 1.1150x; 1.0031x over previous
"""Trainium2 (8 NeuronCores) kernel for a dense causal multi-head attention block.

Problem shapes: B=2, S=2048, D=2048, H=16, DH=128 (fp32 in/out).

Distribution strategy (sharding_hint: tensor-parallel over heads):
  Phase 1 (head parallel): core c owns heads {2c, 2c+1}. It computes, for both
    batches, Q^T/K^T/V^T = W^T @ X^T directly in [DH, S] layout (lhsT = W tile,
    rhs = X^T tile loaded via XBAR DMA-transpose of the bf16 input), then causal
    attention fully on-chip:
       scores^T[k, q] = K^T.T @ Q^T          (PE, one matmul per 128x512 tile)
       p = exp(scores / sqrt(DH))            (ACT, straight from PSUM; no
                                              max-subtraction -- scores are O(1))
       diagonal tiles masked by a 0/1 bf16 mask (DVE)
       z^T[dh, q]  += V_tile.T @ p           (PE, PSUM accumulation over k)
       den[1, q]   += ones.T @ p             (PE rank-reduce for softmax denom)
       z^T *= 1/den (DVE recip + gpsimd partition_broadcast + DVE mult)
  AllToAll (2 MB bf16): reshards z^T from (head-sharded, all rows) to
    (all heads, 512-row shard) so each core owns rows of the final output.
  Phase 2 (row parallel): out[q, d] = Z^T.T @ W_O + b_O for the core's 512 rows.

The host wrapper shards/casts inputs (bf16), runs the SPMD NEFF on cores 0-7,
and concatenates the per-core row slices into the full [2, 2048, 2048] output.
"""

import numpy as np
import ml_dtypes

import concourse.bass as bass
import concourse.mybir as mybir
import concourse.tile as tile
from concourse import bacc
from concourse.bass import ts
from concourse.bass_utils import run_bass_kernel_spmd
from concourse.masks import make_identity

B, S, D, H, DH = 2, 2048, 2048, 16, 128
NCORES = 8
HL = H // NCORES            # heads per core = 2
QB = (B * S) // NCORES      # output rows per core = 512
P = 128
SC = 512                    # free-dim chunk (PSUM bank = 512 fp32)
NSC = S // SC               # 4
NDT = D // P                # 16 contraction tiles for D
NST = S // P                # 16 sequence tiles of 128
NQT = QB // P               # 4 local q tiles in phase 2
NDC = D // SC               # 4 output-dim chunks
SCALE = 1.0 / float(np.sqrt(DH))
MASKW = 384 + SC            # shifted-triangle mask width

F32 = mybir.dt.float32
BF16 = mybir.dt.bfloat16


def build_nc():
    nc = bacc.Bacc("TRN2", target_bir_lowering=False, debug=False,
                   num_devices=NCORES)

    x = nc.dram_tensor("x", [B, S, D], BF16, kind="ExternalInput")
    wq = nc.dram_tensor("wq", [HL, D, DH], BF16, kind="ExternalInput")
    wk = nc.dram_tensor("wk", [HL, D, DH], BF16, kind="ExternalInput")
    wv = nc.dram_tensor("wv", [HL, D, DH], BF16, kind="ExternalInput")
    bq = nc.dram_tensor("bq", [HL, DH], F32, kind="ExternalInput")
    bk = nc.dram_tensor("bk", [HL, DH], F32, kind="ExternalInput")
    bv = nc.dram_tensor("bv", [HL, DH], F32, kind="ExternalInput")
    wo = nc.dram_tensor("wo", [H * DH, D], BF16, kind="ExternalInput")
    bo = nc.dram_tensor("bo", [1, D], BF16, kind="ExternalInput")
    out = nc.dram_tensor("out", [QB, D], F32, kind="ExternalOutput")

    Exp = mybir.ActivationFunctionType.Exp
    Ident = mybir.ActivationFunctionType.Identity

    with tile.TileContext(nc) as tc:
        with (
            tc.tile_pool(name="const", bufs=1) as cpool,
            tc.tile_pool(name="dram", bufs=1, space="DRAM") as dpool,
            tc.tile_pool(name="ps_acc", bufs=3, space="PSUM") as ps_acc,
            tc.tile_pool(name="ps_z", bufs=2, space="PSUM") as ps_z,
            tc.tile_pool(name="ps_vt", bufs=2, space="PSUM") as ps_vt,
            tc.tile_pool(name="ps_den", bufs=1, space="PSUM") as ps_den,
        ):
            # ---- constants ----
            ident = cpool.tile([P, P], BF16)
            make_identity(nc, ident)
            ones_col = cpool.tile([P, 1], BF16)
            nc.vector.memset(ones_col, 1.0)
            ones_row = cpool.tile([1, P], BF16)
            nc.vector.memset(ones_row, 1.0)
            # mask[ki, t] = 1.0 iff ki <= t - 384; slices give the 4 shifted
            # causal triangles needed for the diagonal 128x512 tiles.
            mask = cpool.tile([P, MASKW], BF16)
            nc.gpsimd.memset(mask, 1.0)
            nc.gpsimd.affine_select(
                out=mask, in_=mask, compare_op=mybir.AluOpType.is_ge,
                fill=0.0, base=-384, pattern=[[1, MASKW]], channel_multiplier=-1,
            )
            bias_sb = {}
            for nm, t in (("q", bq), ("k", bk), ("v", bv)):
                bb = cpool.tile([P, HL], F32, tag=f"b{nm}")
                nc.sync.dma_start(bb, t.ap().rearrange("h d -> d h"))
                bias_sb[nm] = bb
            bo_sb = cpool.tile([1, D], BF16)
            nc.sync.dma_start(bo_sb, bo.ap())

            # one AllToAll per local head index: the first launches halfway
            # through phase 1 and hides under compute of the second head.
            a2a_in = [dpool.tile([NCORES, P, SC], BF16, tag=f"a2a_in{hl}",
                                 name=f"a2a_in{hl}") for hl in range(HL)]
            a2a_out = [dpool.tile([NCORES, P, SC], BF16, tag=f"a2a_out{hl}",
                                  name=f"a2a_out{hl}") for hl in range(HL)]

            with (
                tc.tile_pool(name="wpool", bufs=1) as wpool,
                tc.tile_pool(name="xt", bufs=1) as xtpool,
                tc.tile_pool(name="qkv", bufs=2) as qkvpool,
                tc.tile_pool(name="small", bufs=4) as spool,
            ):
                # per-head weight tiles [d_part, d_tile, dh]
                w_sb = []
                for hl in range(HL):
                    per = []
                    for nm, w in (("wq", wq), ("wk", wk), ("wv", wv)):
                        t_sb = wpool.tile([P, NDT, DH], BF16, tag=f"{nm}{hl}")
                        nc.sync.dma_start(
                            t_sb, w.ap()[hl].rearrange("(o p) k -> p o k", p=P))
                        per.append(t_sb)
                    w_sb.append(per)

                XT = {}
                for hl in range(HL):
                    for b in range(B):
                        if hl == 0:
                            # X^T for batch b as 4 per-s-chunk tiles so the
                            # first projection chain starts after ~4MB of
                            # transposed DMA instead of the full 16MB.
                            XT[b] = []
                            for sc in range(NSC):
                                xts = xtpool.tile([P, NDT, SC], BF16,
                                                  tag=f"xt{b}_{sc}",
                                                  name=f"xt{b}_{sc}")
                                for dt_ in range(NDT):
                                    nc.sync.dma_start_transpose(
                                        xts[:, dt_, :],
                                        x.ap()[b][ts(sc, SC), ts(dt_, P)])
                                XT[b].append(xts)

                        # ---- projections: Q^T, K^T, V^T in [dh, s] ----
                        QT = qkvpool.tile([P, S], BF16, tag="qt")
                        KT = qkvpool.tile([P, S], BF16, tag="kt")
                        VT = qkvpool.tile([P, S], BF16, tag="vt")
                        for pi, (dst, bcol) in enumerate((
                            (QT, bias_sb["q"]), (KT, bias_sb["k"]),
                            (VT, bias_sb["v"]),
                        )):
                            wt = w_sb[hl][pi]
                            for sc in range(NSC):
                                ps = ps_acc.tile([P, SC], F32, tag="acc")
                                for dt_ in range(NDT):
                                    nc.tensor.matmul(
                                        ps, lhsT=wt[:, dt_, :],
                                        rhs=XT[b][sc][:, dt_, :],
                                        start=(dt_ == 0), stop=(dt_ == NDT - 1))
                                nc.scalar.activation(
                                    dst[:, ts(sc, SC)], ps, Ident,
                                    bias=bcol[:, hl:hl + 1], scale=1.0)

                        # ---- V in [k, dh] layout via PE transpose ----
                        V_kd = qkvpool.tile([P, NST, DH], BF16, tag="vkd")
                        for st in range(NST):
                            pst = ps_vt.tile([P, P], BF16, tag="vt")
                            nc.tensor.transpose(pst, VT[:, ts(st, P)], ident)
                            nc.scalar.copy(V_kd[:, st, :], pst)

                        # ---- causal attention (scores pipelined 2 ahead so
                        # PE never stalls on ACT exp / DVE mask) ----
                        for qc in range(NSC):
                            z_ps = ps_z.tile([P, SC], F32, tag="z")
                            den_ps = ps_den.tile([1, SC], F32, tag="den")
                            nkt = 4 * qc + 4
                            pexps = {}

                            def emit_scores(kt, qc=qc, pexps=None):
                                s_ps = ps_acc.tile([P, SC], F32, tag="acc")
                                nc.tensor.matmul(
                                    s_ps, lhsT=KT[:, ts(kt, P)],
                                    rhs=QT[:, ts(qc, SC)],
                                    start=True, stop=True)
                                pexp = spool.tile([P, SC], BF16, tag="p")
                                nc.scalar.activation(
                                    pexp, s_ps, Exp, bias=0.0, scale=SCALE)
                                j = kt - 4 * qc
                                if j >= 0:
                                    off = 384 - 128 * j
                                    nc.vector.tensor_mul(
                                        pexp, pexp, mask[:, off:off + SC])
                                pexps[kt] = pexp

                            emit_scores(0, pexps=pexps)
                            if nkt > 1:
                                emit_scores(1, pexps=pexps)
                            for kt in range(nkt):
                                if kt + 2 < nkt:
                                    emit_scores(kt + 2, pexps=pexps)
                                pexp = pexps.pop(kt)
                                nc.tensor.matmul(
                                    z_ps, lhsT=V_kd[:, kt, :], rhs=pexp,
                                    start=(kt == 0), stop=(kt == nkt - 1))
                                nc.tensor.matmul(
                                    den_ps, lhsT=ones_col, rhs=pexp,
                                    start=(kt == 0), stop=(kt == nkt - 1))
                            # normalize: z^T * (1/den) broadcast over partitions
                            rden = spool.tile([1, SC], F32, tag="rden", bufs=2)
                            nc.vector.reciprocal(rden, den_ps)
                            rb = spool.tile([P, SC], F32, tag="rb", bufs=2)
                            nc.gpsimd.partition_broadcast(rb, rden)
                            zs = spool.tile([P, SC], BF16, tag="zs", bufs=2)
                            nc.vector.tensor_mul(zs, z_ps, rb)
                            nc.sync.dma_start(a2a_in[hl][4 * b + qc], zs)

                    # reshard this head's z: all (b, qc) chunks are now queued
                    nc.gpsimd.collective_compute(
                        "AllToAll", mybir.AluOpType.bypass,
                        replica_groups=[list(range(NCORES))],
                        ins=[a2a_in[hl][:]], outs=[a2a_out[hl][:]],
                    )

            # ---- phase 2: output projection for this core's 512 rows ----
            # Split by head parity: even heads (local index 0) arrive with the
            # first AllToAll, so their half of the accumulation overlaps the
            # second collective; odd heads finish and merge.
            with (
                tc.tile_pool(name="p2", bufs=1) as p2pool,
                tc.tile_pool(name="p2o", bufs=2) as p2opool,
            ):
                WO_sb = p2pool.tile([P, H, D], BF16, tag="wo")
                for t in range(H):
                    nc.sync.dma_start(WO_sb[:, t, :], wo.ap()[ts(t, P), :])
                ZT_sb = p2pool.tile([P, H, SC], BF16, tag="zt")
                for j in range(NCORES):
                    nc.sync.dma_start(ZT_sb[:, 2 * j, :], a2a_out[0][j])
                part = {}
                for qt in range(NQT):
                    for dc in range(NDC):
                        ops = ps_acc.tile([P, SC], F32, tag="acc")
                        for j in range(NCORES):
                            nc.tensor.matmul(
                                ops, lhsT=ZT_sb[:, 2 * j, ts(qt, P)],
                                rhs=WO_sb[:, 2 * j, ts(dc, SC)],
                                start=(j == 0), stop=(j == NCORES - 1))
                        pt = p2pool.tile([P, SC], F32, tag=f"part{qt}_{dc}",
                                         name=f"part{qt}_{dc}")
                        nc.scalar.copy(pt, ops)
                        part[qt, dc] = pt
                for j in range(NCORES):
                    nc.sync.dma_start(ZT_sb[:, 2 * j + 1, :], a2a_out[1][j])
                for qt in range(NQT):
                    for dc in range(NDC):
                        ops = ps_acc.tile([P, SC], F32, tag="acc")
                        for j in range(NCORES):
                            nc.tensor.matmul(
                                ops, lhsT=ZT_sb[:, 2 * j + 1, ts(qt, P)],
                                rhs=WO_sb[:, 2 * j + 1, ts(dc, SC)],
                                start=(j == 0), stop=False)
                        nc.tensor.matmul(
                            ops, lhsT=ones_row, rhs=bo_sb[:, ts(dc, SC)],
                            start=False, stop=True)
                        osb = p2opool.tile([P, SC], F32, tag="osb")
                        nc.vector.tensor_add(osb, ops, part[qt, dc])
                        nc.sync.dma_start(out.ap()[ts(qt, P), ts(dc, SC)], osb)

    nc.compile()
    return nc


_CACHE = {}


def _get_nc():
    if "nc" not in _CACHE:
        _CACHE["nc"] = build_nc()
    return _CACHE["nc"]


def make_in_maps(resid_pre, W_Q, W_K, W_V, W_O, b_Q, b_K, b_V, b_O):
    bf = ml_dtypes.bfloat16
    x = np.ascontiguousarray(np.asarray(resid_pre, np.float32)).astype(bf)
    WQ = np.asarray(W_Q, np.float32)
    WK = np.asarray(W_K, np.float32)
    WV = np.asarray(W_V, np.float32)
    WOf = np.ascontiguousarray(
        np.asarray(W_O, np.float32).reshape(H * DH, D)).astype(bf)
    bQ = np.ascontiguousarray(np.asarray(b_Q, np.float32))
    bK = np.ascontiguousarray(np.asarray(b_K, np.float32))
    bV = np.ascontiguousarray(np.asarray(b_V, np.float32))
    bO = np.ascontiguousarray(np.asarray(b_O, np.float32)).reshape(1, D).astype(bf)
    in_maps = []
    for c in range(NCORES):
        hs = slice(c * HL, (c + 1) * HL)
        in_maps.append({
            "x": x,
            "wq": np.ascontiguousarray(WQ[hs]).astype(bf),
            "wk": np.ascontiguousarray(WK[hs]).astype(bf),
            "wv": np.ascontiguousarray(WV[hs]).astype(bf),
            "bq": bQ[hs].copy(),
            "bk": bK[hs].copy(),
            "bv": bV[hs].copy(),
            "wo": WOf,
            "bo": bO,
        })
    return in_maps


def assemble(results):
    out = np.empty((B, S, D), np.float32)
    for c in range(NCORES):
        b, r = divmod(c, NCORES // B)  # divmod(c, 4)
        out[b, r * QB:(r + 1) * QB] = results[c]["out"]
    return out


def kernel(resid_pre, W_Q, W_K, W_V, W_O, b_Q, b_K, b_V, b_O,
           _trace=False, _return_raw=False):
    nc = _get_nc()
    in_maps = make_in_maps(resid_pre, W_Q, W_K, W_V, W_O, b_Q, b_K, b_V, b_O)
    res = run_bass_kernel_spmd(nc, in_maps, core_ids=list(range(NCORES)),
                               trace=_trace)
    out = assemble(res.results)
    if _return_raw:
        return out, res
    return out
